# revision 3
# baseline (speedup 1.0000x reference)
# Trainium2 Bass kernel for streaming weighted DTW features.
#
# reference recurrence (per batch b, pattern p):
#   D[i,j] = cost[i,j] + min(D[i-1,j], w*D[i,j-1], w*D[i-1,j-1])
#   D[i,0] = cumsum_i cost[i,0];  out[b,p,j] = sqrt(D[L-1,j])
#   cost[i,j] = ||x[b,:,j] - patts[p,:,i]||^2
#
# Device formulation: substitute V[i,j] = D[i,j] * w^(-j).  Then
#   V[i,j] = c'[i,j] + min(V[i-1,j], V[i,j-1], V[i-1,j-1]),
#   c'[i,j] = cost[i,j] * w^(-j)
# i.e. a plain unweighted DTW on rescaled costs -> per time column j:
#   m[i]   = min(V[i,j-1], V[i-1,j-1])            (one tensor_tensor min)
#   V[:,j] = scan_i: state = min(m[i], state) + c'[i,j]   (one tensor_tensor_scan)
# The rescaled costs come straight out of the PE via an augmented matmul:
#   lhsT rows 0..15 = patts, row 16 = ||patts||^2, row 17 = 1
#   rhs  rows 0..15 = -2*x*w^(-t), row 16 = w^(-t), row 17 = ||x||^2*w^(-t)
# Sharding: data-parallel over batch, 4 batches per core x 8 cores.
# Per-core layout: partition = b_in*64 + p (b_in in {0,1}), the other two
# batches ride in the free dim as a second 32-row group separated by a
# BIG cost row, so one scan instruction covers all 256 (b,p) problems.

import numpy as np

B, D, T = 32, 16, 1024
P, L = 64, 32
NCORE = 8
BLOC = B // NCORE          # 4 batches per core
K = D + 2                  # 18 contraction rows (patts, p2, ones)
Tc = 128                   # time-chunk size
NCH = T // Tc
CB = 2 * L + 1             # 65 cost rows: [bg0 l0..31][SEP][bg1 l0..31]
RB = CB + 1                # 66 V rows (leading BIG pad row)
VC = Tc + 1                # V history cols (col 0 = prev chunk's last col)
BIG = 1e30

_NC_CACHE = {}


def _install_multiwait_fix():
    """This container's walrus codegen rejects instructions carrying more
    than one semaphore wait (Tile emits those).  Split extra waits into
    standalone EventSemaphore instructions at the BIR-JSON level."""
    import json
    import concourse.bass2jax as bass2jax
    import concourse.bass_utils as bass_utils

    if getattr(bass2jax.compile_bir_kernel, "_is_multiwait_fix", False):
        return
    orig = bass_utils.compile_bir_kernel
    ctr = [0]

    def legalize(bir_json: bytes) -> bytes:
        d = json.loads(bir_json)
        changed = [False]

        def fix(block):
            newinsts = []
            for inst in block.get("instructions", []):
                s = inst.get("sync_info")
                if s and len(s.get("on_wait", [])) > 1:
                    changed[0] = True
                    waits = s["on_wait"]
                    for wcond in waits[:-1]:
                        ctr[0] += 1
                        newinsts.append({
                            "debug": inst.get("debug", 0),
                            "engine": inst["engine"],
                            "ins": [], "outs": [],
                            "name": f"mwfix-{ctr[0]}",
                            "opcode": "EventSemaphore",
                            "sync_info": {"on_update": [], "on_wait": [wcond]},
                        })
                    s["on_wait"] = [waits[-1]]
                newinsts.append(inst)
            block["instructions"] = newinsts
            for sub in block.get("blocks", []):
                fix(sub)

        for f in d["functions"]:
            for blk in f["blocks"]:
                fix(blk)
        return json.dumps(d).encode() if changed[0] else bir_json

    def patched(bir_json, tmpdir, neff_name="file.neff"):
        return orig(legalize(bir_json), tmpdir, neff_name)

    patched._is_multiwait_fix = True
    bass2jax.compile_bir_kernel = patched
    bass_utils.compile_bir_kernel = patched


def _build_nc():
    import concourse.bass as bass
    import concourse.tile as tile
    from concourse import mybir

    F32 = mybir.dt.float32
    AL = mybir.AluOpType
    nc = bass.Bass("TRN2", target_bir_lowering=False, debug=False,
                   num_devices=NCORE)
    lhsT_t = nc.dram_tensor("lhsT", [K, P * L], F32, kind="ExternalInput")
    rhs_t = nc.dram_tensor("rhs", [K, BLOC * T], F32, kind="ExternalInput")
    out_t = nc.dram_tensor("out", [128, 2 * T], F32, kind="ExternalOutput")

    with tile.TileContext(nc, num_cores=NCORE) as tc:
        with tc.tile_pool(name="const", bufs=1) as cp, \
             tc.tile_pool(name="psum", bufs=8, space="PSUM") as pp:
            lhsT = cp.tile([K, P * L], F32, tag="lhsT")
            rhs = cp.tile([K, BLOC * T], F32, tag="rhs")
            vh = cp.tile([128, RB * VC], F32, tag="vh")
            mb = cp.tile([128, CB], F32, tag="mb")
            costs = [cp.tile([128, CB * Tc], F32, name=f"cost{i}",
                             tag=f"cost{i}") for i in range(2)]

            nc.sync.dma_start(lhsT[:], lhsT_t.ap()[:])
            nc.sync.dma_start(rhs[:], rhs_t.ap()[:])
            nc.vector.memset(vh[:], BIG)
            nc.vector.memset(mb[:], BIG)
            cost3 = [c[:].rearrange("p (r t) -> p r t", r=CB) for c in costs]
            for i in range(2):
                nc.gpsimd.memset(cost3[i][:, L, :], BIG)   # SEP cost row
            vh3 = vh[:].rearrange("p (r c) -> p r c", r=RB)
            out2 = out_t.ap().rearrange("p (g t) -> p g t", g=2)

            for c in range(NCH):
                cb3 = cost3[c % 2]
                # produce this chunk's rescaled costs on the PE
                for l in range(L):
                    for bg in range(2):
                        pt = pp.tile([128, Tc], F32)
                        for b_in in range(2):
                            bl = bg * 2 + b_in
                            nc.tensor.matmul(
                                pt[b_in * 64:(b_in + 1) * 64, :],
                                lhsT[:, l * P:(l + 1) * P],
                                rhs[:, bl * T + c * Tc: bl * T + (c + 1) * Tc],
                                start=True, stop=True)
                        li = l + (L + 1) * bg
                        nc.scalar.copy(cb3[:, li, :], pt[:, :])
                # consume: one column per (min, scan) pair on DVE
                for k_ in range(Tc):
                    j = c * Tc + k_
                    if j == 0:
                        # column 0 is a plain cumsum (init 0, m stays BIG)
                        nc.vector.tensor_tensor_scan(
                            vh3[:, 1:1 + L, 1], mb[:, 0:L], cb3[:, 0:L, 0],
                            0.0, AL.min, AL.add)
                        nc.vector.tensor_tensor_scan(
                            vh3[:, L + 2:RB, 1], mb[:, L + 1:CB],
                            cb3[:, L + 1:CB, 0], 0.0, AL.min, AL.add)
                    else:
                        nc.vector.tensor_tensor(
                            mb[:], vh3[:, 1:RB, k_], vh3[:, 0:RB - 1, k_],
                            AL.min)
                        nc.vector.tensor_tensor_scan(
                            vh3[:, 1:RB, k_ + 1], mb[:], cb3[:, :, k_],
                            BIG, AL.min, AL.add)
                # stream out last-row values (V[L-1, :]) for both groups
                nc.sync.dma_start(out2[:, 0, c * Tc:(c + 1) * Tc],
                                  vh3[:, L, 1:VC])
                nc.sync.dma_start(out2[:, 1, c * Tc:(c + 1) * Tc],
                                  vh3[:, RB - 1, 1:VC])
                if c < NCH - 1:
                    nc.scalar.copy(vh3[:, :, 0], vh3[:, :, Tc])
    return nc


def _get_nc():
    if "nc" not in _NC_CACHE:
        _install_multiwait_fix()
        _NC_CACHE["nc"] = _build_nc()
    return _NC_CACHE["nc"]


def _prep_inputs(x, patts, w):
    x64 = np.asarray(x, dtype=np.float64)
    p64 = np.asarray(patts, dtype=np.float64)
    t_idx = np.arange(T, dtype=np.float64)
    s = w ** (-t_idx)                                   # w^-t
    p2 = (p64 * p64).sum(axis=1)                        # (P, L)
    x2 = (x64 * x64).sum(axis=1)                        # (B, T)

    lhsT = np.empty((K, P * L), np.float32)
    lhsT[:D] = p64.transpose(1, 2, 0).reshape(D, L * P)  # col = l*P + p
    lhsT[D] = p2.T.reshape(L * P)
    lhsT[D + 1] = 1.0

    in_maps = []
    for ci in range(NCORE):
        rhs = np.empty((K, BLOC, T), np.float64)
        for bl in range(BLOC):
            b = ci * BLOC + bl
            rhs[:D, bl] = -2.0 * x64[b] * s[None, :]
            rhs[D, bl] = s
            rhs[D + 1, bl] = x2[b] * s
        in_maps.append({"lhsT": lhsT,
                        "rhs": rhs.reshape(K, BLOC * T).astype(np.float32)})
    return in_maps


def _postprocess(results, w):
    t_idx = np.arange(T, dtype=np.float64)
    wj = w ** t_idx
    V = np.empty((B, P, T), np.float64)
    for ci in range(NCORE):
        o = results[ci]["out"].reshape(2, 64, 2, T).astype(np.float64)
        for bg in range(2):
            for b_in in range(2):
                V[ci * BLOC + bg * 2 + b_in] = o[b_in, :, bg, :]
    dtw = V * wj[None, None, :]
    return np.sqrt(np.maximum(dtw, 0.0)).astype(np.float32)


def kernel(x, patts, w):
    import concourse.bass_utils as bass_utils
    w = float(w)
    _install_multiwait_fix()
    in_maps = _prep_inputs(x, patts, w)
    nc = _get_nc()
    res = bass_utils.run_bass_kernel_spmd(nc, in_maps,
                                          core_ids=list(range(NCORE)))
    return _postprocess(res.results, w)
